# revision 10
# baseline (speedup 1.0000x reference)
"""StyleGAN2-style modulated 3x3 conv (B=8, Ci=Co=512, H=W=32) on 8 TRN2
NeuronCores, via 1-D Winograd F(4,3) along y, fp16 compute.

Sharding: data-parallel over batch, one sample per core (embarrassingly
parallel, no collectives).

Per core the conv is decomposed with 1-D Winograd F(4,3) applied to the
ky axis: the 3 ky taps collapse into 6 transform phases a=0..5, each
y-tile producing 4 output rows — 2x fewer MACs than direct conv
(288 N=256 matmuls instead of 288 N=512):

  V_a[ty, x'] = sum_r Bt[a,r] * pad[4*ty + r, x']            (DVE fp16)
  M_a[co]     = sum_{kx,ci} U1[a,kx,ci,co] V_a[ci][:, kx:kx+32]   (PE)
  out[4ty+p]  = (sum_a At[p,a] M_a) * rs + bias

with
  Bt = [[4,0,-5,0,1,0],[0,-4,-4,1,1,0],[0,4,-4,-1,1,0],
        [0,-2,-1,2,1,0],[0,2,-1,-2,1,0],[0,4,0,-5,0,1]]
  At = [[1,1,1,1,1,0],[0,1,-1,2,-2,0],[0,1,1,4,4,0],[0,1,-1,8,-8,1]]
  U1[a,kx] = sum_ky G[a,ky] w[:,:,ky,kx]
  G  = [[1/4,0,0],[-1/6,-1/6,-1/6],[-1/6,1/6,-1/6],
        [1/24,1/12,1/6],[1/24,-1/12,1/6],[0,0,1]]

U1 is an input-independent weight transform folded on the host (same
category as the baseline's w2 fold); all input-dependent math runs on
device. fp16 (10 mantissa bits) absorbs the Winograd transforms'
amplification — simulated rel err 2.2e-3 vs 2.7e-3 for the direct
bf16 conv.

Scheduling: matmul groups run a-MAJOR (a0 for all 4 co-tiles, then a1,
...) so that (1) each V level is consumed over ~5us while the DVE
produces the next in ~2.6us, and (2) the 10MB weight stream is
consumed evenly at ~280 GB/s < the 358 GB/s HBM limit. Weights are
shipped as per-(co-tile, a) chunks whose DMA order matches consumption
order exactly, throttled to ~2 transfers in flight (the rings
round-robin across active transfers, so a burst would make the first-
needed chunk land as late as the last). Per-co-tile epilogues
(demod + A-combine + scale/bias + store) are emitted inside the a5
level so only the last co-tile's epilogue sits on the critical tail.

Demod as in the baseline: conv runs on raw (unscaled) weights and the
per-(b,co) norm uses w2 = sum_k w^2 with compensated eps:
  out = conv / sqrt(sum_ci ys^2 * w2 + 1e-8*Ci*K^2) + bias
"""

import numpy as np

import concourse.mybir as mybir
from concourse import bacc
from concourse.tile import TileContext
from concourse.bass_utils import run_bass_kernel_spmd

B = 8
CI = 512
CO = 512
H = W = 32
NCI = CI // 128
NCO = CO // 128
ALPHA = 6          # F(4,3): 6 transform phases
MOUT = 4           # output rows per tile
NTY = H // MOUT    # 8 y-tiles
PADH = 34
PADW = 36          # cols: [0..1]=left border, [2..33]=x, [34..35]=right border
EPS_EFF = 1e-8 * CI * 9

F32 = mybir.dt.float32
F16 = mybir.dt.float16
AF = mybir.ActivationFunctionType
ALU = mybir.AluOpType


def build_nc():
    nc = bacc.Bacc("TRN2", target_bir_lowering=False, debug=False)

    x_ext = nc.declare_dram_parameter("x", [NCI, 128, H, W], F16, isOutput=False)
    # cols 0..3 = y_s per ci-tile, cols 4..7 = bias per co-tile
    yb_ext = nc.declare_dram_parameter("yb", [128, 2 * NCI], F32, isOutput=False)
    # conv weights as per-(jo, a) chunks: [jo, a, ci_p, jci, kx, co]
    wa_ext = nc.declare_dram_parameter(
        "wa", [NCO, ALPHA, 128, NCI, 3, 128], F16, isOutput=False
    )
    # demod weights: [jo, ci_p, jci, co]
    w2_ext = nc.declare_dram_parameter(
        "w2", [NCO, 128, NCI, 128], F16, isOutput=False
    )
    out_ext = nc.declare_dram_parameter("out", [NCO, 128, H * W], F16, isOutput=True)

    with TileContext(nc) as tc:
        with (
            tc.tile_pool(name="singles", bufs=1) as singles,
            tc.tile_pool(name="wts", bufs=1) as wts,
            tc.tile_pool(name="pads", bufs=1) as pads,
            tc.tile_pool(name="vts", bufs=1) as vts,
            tc.tile_pool(name="vtmp", bufs=1) as vtmp,
            tc.tile_pool(name="xin", bufs=4) as xin,
            tc.tile_pool(name="mbs", bufs=26) as mbs,
            tc.tile_pool(name="zts", bufs=2) as zts,
            tc.tile_pool(name="outs", bufs=2) as outs,
            tc.tile_pool(name="cps", bufs=6, space="PSUM") as cps,
            tc.tile_pool(name="dps", bufs=1, space="PSUM") as dps,
            tc.tile_pool(name="wps", bufs=1, space="PSUM") as wps,
        ):
            # ---- input DMAs ----
            # x0/x1 + yb from sync, x2/x3 from gpsimd — 4 concurrent x
            # transfers land everything by ~4us while the first weight
            # chunks stream alongside.
            xt_sb = [
                xin.tile([128, H, W], F16, tag=f"x{j}", name=f"xt{j}")
                for j in range(NCI)
            ]
            yb_sb = singles.tile([128, 2 * NCI], F32)
            nc.sync.dma_start(out=xt_sb[0], in_=x_ext[0])
            nc.sync.dma_start(out=yb_sb, in_=yb_ext[:, :])
            nc.sync.dma_start(out=xt_sb[1], in_=x_ext[1])

            # weight chunk stream: consumption order is (a0: jo0..3),
            # (a1: jo0..3), ..., (a4: jo0..3), w2 jo0..3, (a5: jo0..3)
            wa_sb = [[None] * ALPHA for _ in range(NCO)]
            w2_sb = [None] * NCO
            chunk_order = []
            for a in range(ALPHA - 1):
                for jo in range(NCO):
                    chunk_order.append((jo, a))
            for jo in range(NCO):
                chunk_order.append((jo, -1))  # w2
            for jo in range(NCO):
                chunk_order.append((jo, ALPHA - 1))

            def wdma(c):
                jo, a = c
                if a < 0:
                    w = wts.tile([128, NCI, 128], F16, tag=f"w2_{jo}", name=f"w2_{jo}")
                    nc.gpsimd.dma_start(out=w, in_=w2_ext[jo])
                    w2_sb[jo] = w
                else:
                    w = wts.tile(
                        [128, NCI, 3, 128], F16, tag=f"wa{jo}_{a}", name=f"wa{jo}_{a}"
                    )
                    nc.gpsimd.dma_start(out=w, in_=wa_ext[jo, a])
                    wa_sb[jo][a] = w

            def chunk_probe(c):
                jo, a = c
                t = w2_sb[jo] if a < 0 else wa_sb[jo][a]
                return t[0:1, 0, 0:1] if a < 0 else t[0:1, 0, 0, 0:1]

            wscr = singles.tile([1, 1], F16)

            wdma(chunk_order[0])
            wdma(chunk_order[1])
            nc.gpsimd.dma_start(out=xt_sb[2], in_=x_ext[2])
            nc.gpsimd.dma_start(out=xt_sb[3], in_=x_ext[3])
            for i in range(2, len(chunk_order)):
                nc.gpsimd.tensor_copy(out=wscr, in_=chunk_probe(chunk_order[i - 2]))
                wdma(chunk_order[i])

            def wa_slice(jo, jci, a, kx):
                return wa_sb[jo][a][:, jci, kx, :]

            # ---- PE warm-up: throwaway matmuls on memset data so the
            # HAM clock gate starts releasing before the real stream ----
            warm_lhs = singles.tile([128, 1], F16)
            nc.vector.memset(warm_lhs, 1.0)
            warm_rhs = singles.tile([128, 512], F16)
            nc.vector.memset(warm_rhs, 0.5)
            warm_ps = wps.tile([1, 512], F32)
            N_WARM = 8
            for i in range(N_WARM):
                nc.tensor.matmul(
                    out=warm_ps,
                    lhsT=warm_lhs,
                    rhs=warm_rhs,
                    start=(i == 0),
                    stop=(i == N_WARM - 1),
                )

            eps_sb = singles.tile([128, 1], F32)
            nc.vector.memset(eps_sb, EPS_EFF)

            # pad border memsets on DVE (fast, and DVE owns pad deps
            # anyway; gpsimd is busy issuing the weight chain)
            pad_sb = []
            for j in range(NCI):
                p = pads.tile([128, PADH, PADW], F16, tag=f"pad{j}")
                nc.vector.memset(p[:, 0, :], 0.0)
                nc.vector.memset(p[:, PADH - 1, :], 0.0)
                nc.vector.memset(p[:, 1 : PADH - 1, 0:2], 0.0)
                nc.vector.memset(p[:, 1 : PADH - 1, PADW - 2 : PADW], 0.0)
                pad_sb.append(p)

            # modulate on ACT (frees the DVE for the V transform)
            for j in range(NCI):
                nc.scalar.activation(
                    out=pad_sb[j][:, 1 : H + 1, 2 : W + 2],
                    in_=xt_sb[j],
                    func=AF.Identity,
                    scale=yb_sb[:, j : j + 1],
                )

            # ---- F(4,3) y-transform (DVE fp16), emitted a-MAJOR so
            # V levels become ready in consumption order ----
            v_sb = [[None] * NCI for _ in range(ALPHA)]
            for a in range(ALPHA):
                for j in range(NCI):
                    v_sb[a][j] = vts.tile(
                        [128, NTY, PADW], F16, tag=f"v{a}_{j}", name=f"v{a}_{j}"
                    )
            P_ = lambda j, r: pad_sb[j][:, r : r + 29 : 4, :]

            def vtile(nm, j):
                return vtmp.tile([128, NTY, PADW], F16, tag=f"{nm}{j}", name=f"{nm}{j}")

            stt = nc.vector.scalar_tensor_tensor
            tmps = [dict() for _ in range(NCI)]
            for j in range(NCI):  # V0 = 4*P0 - 5*P2 + P4
                t0 = vtile("t0", j)
                stt(out=t0, in0=P_(j, 2), scalar=-5.0, in1=P_(j, 4), op0=ALU.mult, op1=ALU.add)
                stt(out=v_sb[0][j], in0=P_(j, 0), scalar=4.0, in1=t0, op0=ALU.mult, op1=ALU.add)
            for j in range(NCI):  # V1 = -4*(P1+P2) + (P3+P4)
                s12 = vtile("s12", j)
                s34 = vtile("s34", j)
                nc.vector.tensor_add(s12, P_(j, 1), P_(j, 2))
                nc.vector.tensor_add(s34, P_(j, 3), P_(j, 4))
                stt(out=v_sb[1][j], in0=s12, scalar=-4.0, in1=s34, op0=ALU.mult, op1=ALU.add)
            for j in range(NCI):  # V2 = 4*(P1-P2) + (P4-P3)
                d12 = vtile("d12", j)
                d43 = vtile("d43", j)
                nc.vector.tensor_sub(d12, P_(j, 1), P_(j, 2))
                nc.vector.tensor_sub(d43, P_(j, 4), P_(j, 3))
                stt(out=v_sb[2][j], in0=d12, scalar=4.0, in1=d43, op0=ALU.mult, op1=ALU.add)
            for j in range(NCI):  # V3 = 2*(P3-P1) + (P4-P2)
                d31 = vtile("d31", j)
                d42 = vtile("d42", j)
                nc.vector.tensor_sub(d31, P_(j, 3), P_(j, 1))
                nc.vector.tensor_sub(d42, P_(j, 4), P_(j, 2))
                stt(out=v_sb[3][j], in0=d31, scalar=2.0, in1=d42, op0=ALU.mult, op1=ALU.add)
                tmps[j]["d31"] = d31
                tmps[j]["d42"] = d42
            for j in range(NCI):  # V4 = -2*(P3-P1) + (P4-P2)
                stt(out=v_sb[4][j], in0=tmps[j]["d31"], scalar=-2.0, in1=tmps[j]["d42"], op0=ALU.mult, op1=ALU.add)
            for j in range(NCI):  # V5 = 4*P1 - 5*P3 + P5
                t5 = vtile("t5", j)
                stt(out=t5, in0=P_(j, 3), scalar=-5.0, in1=P_(j, 5), op0=ALU.mult, op1=ALU.add)
                stt(out=v_sb[5][j], in0=P_(j, 1), scalar=4.0, in1=t5, op0=ALU.mult, op1=ALU.add)

            # ys^2 in fp16 for the demod matmuls
            ys2_sb = singles.tile([128, NCI], F16)
            nc.vector.tensor_mul(ys2_sb, yb_sb[:, 0:NCI], yb_sb[:, 0:NCI])

            xs2_ps = dps.tile([128, NCO], F32)
            rs_sb = singles.tile([128, NCO], F32)
            mb = [[None] * ALPHA for _ in range(NCO)]

            def epilogue(jo):
                # demod
                for jci in range(NCI):
                    nc.tensor.matmul(
                        out=xs2_ps[:, jo : jo + 1],
                        lhsT=w2_sb[jo][:, jci, :],
                        rhs=ys2_sb[:, jci : jci + 1],
                        start=(jci == 0),
                        stop=(jci == NCI - 1),
                    )
                nc.scalar.activation(
                    out=rs_sb[:, jo : jo + 1],
                    in_=xs2_ps[:, jo : jo + 1],
                    func=AF.Sqrt,
                    bias=eps_sb,
                )
                nc.vector.reciprocal(
                    out=rs_sb[:, jo : jo + 1], in_=rs_sb[:, jo : jo + 1]
                )
                # combine (DVE fp16):
                #   p0 = M0+M1+M2+M3+M4, p1 = (M1-M2) + 2(M3-M4)
                #   p2 = (M1+M2) + 4(M3+M4), p3 = ((M1-M2)+M5) + 8(M3-M4)
                def zt(tag):
                    return zts.tile([128, NTY, W], F16, tag=tag, name=tag)

                m = mb[jo]
                sp, sm, tp, tm = zt("sp"), zt("sm"), zt("tp"), zt("tm")
                nc.vector.tensor_add(sp, m[1], m[2])
                nc.vector.tensor_sub(sm, m[1], m[2])
                nc.vector.tensor_add(tp, m[3], m[4])
                nc.vector.tensor_sub(tm, m[3], m[4])
                u0, z0, z1, z2, z3, u3 = (
                    zt("u0"), zt("z0"), zt("z1"), zt("z2"), zt("z3"), zt("u3"),
                )
                nc.vector.tensor_add(u0, m[0], sp)
                nc.vector.tensor_add(z0, u0, tp)
                stt(out=z1, in0=tm, scalar=2.0, in1=sm, op0=ALU.mult, op1=ALU.add)
                stt(out=z2, in0=tp, scalar=4.0, in1=sp, op0=ALU.mult, op1=ALU.add)
                nc.vector.tensor_add(u3, sm, m[5])
                stt(out=z3, in0=tm, scalar=8.0, in1=u3, op0=ALU.mult, op1=ALU.add)
                # out rows 4ty+p = Z_p * rs + bias
                ot = outs.tile([128, H, W], F16, tag="ot", name="ot")
                for p, z in ((0, z0), (1, z1), (2, z2), (3, z3)):
                    nc.scalar.activation(
                        out=ot[:, p : p + 29 : 4, :],
                        in_=z,
                        func=AF.Identity,
                        bias=yb_sb[:, NCI + jo : NCI + jo + 1],
                        scale=rs_sb[:, jo : jo + 1],
                    )
                nc.sync.dma_start(out=out_ext[jo], in_=ot)

            # ---- main stream: a-major groups; per-jo epilogues are
            # emitted inside the a5 level right after each jo's last
            # group so only jo3's epilogue trails the stream ----
            for a in range(ALPHA):
                for jo in range(NCO):
                    ps = cps.tile([128, NTY, W], F32, tag="m", name="m")
                    idx = 0
                    for jci in range(NCI):
                        for kx in range(3):
                            nc.tensor.matmul(
                                out=ps,
                                lhsT=wa_slice(jo, jci, a, kx),
                                rhs=v_sb[a][jci][:, :, kx + 1 : kx + 1 + W],
                                start=(idx == 0),
                                stop=(idx == 11),
                            )
                            idx += 1
                    m = mbs.tile([128, NTY, W], F16, tag="mb", name="mb")
                    nc.scalar.activation(out=m, in_=ps, func=AF.Copy)
                    mb[jo][a] = m
                    if a == ALPHA - 1:
                        epilogue(jo)

            # keep the warm-up matmuls live (cheap PSUM read at the end)
            warm_sink = singles.tile([1, 1], F32)
            nc.vector.tensor_copy(out=warm_sink, in_=warm_ps[0:1, 0:1])
    nc.compile()
    return nc


_NC_CACHE = None


def _get_nc():
    global _NC_CACHE
    if _NC_CACHE is None:
        _NC_CACHE = build_nc()
    return _NC_CACHE


def _prep_inputs(x, y_s, weight, bias):
    # Winograd weight transform (input-independent): U1[a,kx,ci,co] =
    # sum_ky G[a,ky] w[co,ci,ky,kx]; w2 = sum_k w^2 for demod.
    G = np.array(
        [
            [1 / 4, 0, 0],
            [-1 / 6, -1 / 6, -1 / 6],
            [-1 / 6, 1 / 6, -1 / 6],
            [1 / 24, 1 / 12, 1 / 6],
            [1 / 24, -1 / 12, 1 / 6],
            [0, 0, 1],
        ],
        np.float64,
    )
    w64 = weight.astype(np.float64)  # [co, ci, ky, kx]
    u1 = np.einsum("ag,oigx->axio", G, w64)  # [a, kx, ci, co]
    # -> [jo, a, ci_p, jci, kx, co]
    wa = np.ascontiguousarray(
        u1.reshape(ALPHA, 3, NCI, 128, NCO, 128).transpose(4, 0, 3, 2, 1, 5)
    ).astype(np.float16)
    w2 = (w64**2).sum(axis=(2, 3)).T  # [ci, co]
    w2q = np.ascontiguousarray(
        w2.reshape(NCI, 128, NCO, 128).transpose(2, 1, 0, 3)
    ).astype(np.float16)
    in_maps = []
    for b in range(B):
        yb = np.empty((128, 2 * NCI), np.float32)
        yb[:, :NCI] = y_s[b].reshape(NCI, 128).T
        yb[:, NCI:] = bias.reshape(NCO, 128).T
        in_maps.append(
            {
                "x": np.ascontiguousarray(x[b].reshape(NCI, 128, H, W)).astype(
                    np.float16
                ),
                "yb": yb,
                "wa": wa,
                "w2": w2q,
            }
        )
    return in_maps


def _install_trace_support():
    """Dev-only: register the axon NTFF profiling hook + disable the
    remote artifact upload so trace=True works in this container."""
    import sys
    import types

    import concourse.bass_utils as bu

    bu.upload_artifacts = lambda tmpdir: "local://" + str(tmpdir)
    if "antenv.axon_hooks" in sys.modules:
        return
    try:
        from trn_agent_boot.trn_boot import _ntff_profile_via_ctypes

        hook = _ntff_profile_via_ctypes("/opt/axon/libaxon_pjrt.so")
    except Exception:
        return
    mod = types.ModuleType("antenv.axon_hooks")
    mod.get_axon_ntff_profile_hook = lambda: hook
    mod.set_axon_ntff_profile_hook = lambda h: None
    sys.modules["antenv.axon_hooks"] = mod


def run(x, y_s, weight, bias, trace=False, tmpdir=None):
    nc = _get_nc()
    if trace:
        _install_trace_support()
    in_maps = _prep_inputs(x, y_s, weight, bias)
    res = run_bass_kernel_spmd(
        nc, in_maps, core_ids=list(range(B)), trace=trace, tmpdir=tmpdir
    )
    out = np.stack(
        [res.results[b]["out"].reshape(CO, H, W).astype(np.float32) for b in range(B)]
    )
    return out, res


def kernel(x, y_s, weight, bias):
    out, _ = run(
        np.asarray(x, dtype=np.float32),
        np.asarray(y_s, dtype=np.float32),
        np.asarray(weight, dtype=np.float32),
        np.asarray(bias, dtype=np.float32),
    )
    return out


# revision 13
# speedup vs baseline: 1.0658x; 1.0658x over previous
"""StyleGAN2-style modulated 3x3 conv (B=8, Ci=Co=512, H=W=32) on 8 TRN2
NeuronCores, via 1-D Winograd F(4,3) along y, fp16 compute.

Sharding: data-parallel over batch, one sample per core (embarrassingly
parallel, no collectives).

Per core the conv is decomposed with 1-D Winograd F(4,3) applied to the
ky axis: the 3 ky taps collapse into 6 transform phases a=0..5, each
y-tile producing 4 output rows — 2x fewer MACs than direct conv
(288 N=256 matmuls instead of 288 N=512):

  V_a[ty, x'] = sum_r Bt[a,r] * pad[4*ty + r, x']            (DVE fp16)
  M_a[co]     = sum_{kx,ci} U1[a,kx,ci,co] V_a[ci][:, kx:kx+32]   (PE)
  out[4ty+p]  = (sum_a At[p,a] M_a) * rs + bias

with
  Bt = [[4,0,-5,0,1,0],[0,-4,-4,1,1,0],[0,4,-4,-1,1,0],
        [0,-2,-1,2,1,0],[0,2,-1,-2,1,0],[0,4,0,-5,0,1]]
  At = [[1,1,1,1,1,0],[0,1,-1,2,-2,0],[0,1,1,4,4,0],[0,1,-1,8,-8,1]]
  U1[a,kx] = sum_ky G[a,ky] w[:,:,ky,kx]
  G  = [[1/4,0,0],[-1/6,-1/6,-1/6],[-1/6,1/6,-1/6],
        [1/24,1/12,1/6],[1/24,-1/12,1/6],[0,0,1]]

U1 is an input-independent weight transform folded on the host (same
category as the baseline's w2 fold); all input-dependent math runs on
device. fp16 (10 mantissa bits) absorbs the Winograd transforms'
amplification — simulated rel err 2.2e-3 vs 2.7e-3 for the direct
bf16 conv.

Scheduling: matmul groups run a-MAJOR (a0 for all 4 co-tiles, then a1,
...) so that (1) each V level is consumed over ~5us while the DVE
produces the next in ~2.6us, and (2) the 10MB weight stream is
consumed evenly at ~280 GB/s < the 358 GB/s HBM limit. Weights are
shipped as per-(co-tile, a) chunks whose DMA order matches consumption
order exactly, throttled to ~2 transfers in flight (the rings
round-robin across active transfers, so a burst would make the first-
needed chunk land as late as the last). Per-co-tile epilogues
(demod + A-combine + scale/bias + store) are emitted inside the a5
level so only the last co-tile's epilogue sits on the critical tail.

Demod as in the baseline: conv runs on raw (unscaled) weights and the
per-(b,co) norm uses w2 = sum_k w^2 with compensated eps:
  out = conv / sqrt(sum_ci ys^2 * w2 + 1e-8*Ci*K^2) + bias
"""

import numpy as np

import concourse.mybir as mybir
from concourse import bacc
from concourse.tile import TileContext
from concourse.bass_utils import run_bass_kernel_spmd

B = 8
CI = 512
CO = 512
H = W = 32
NCI = CI // 128
NCO = CO // 128
ALPHA = 6          # F(4,3): 6 transform phases
MOUT = 4           # output rows per tile
NTY = H // MOUT    # 8 y-tiles
PADH = 34
PADW = 36          # cols: [0..1]=left border, [2..33]=x, [34..35]=right border
EPS_EFF = 1e-8 * CI * 9

F32 = mybir.dt.float32
F16 = mybir.dt.float16
AF = mybir.ActivationFunctionType
ALU = mybir.AluOpType


def build_nc():
    nc = bacc.Bacc("TRN2", target_bir_lowering=False, debug=False)

    x_ext = nc.declare_dram_parameter("x", [NCI, 128, H, W], F16, isOutput=False)
    # cols 0..3 = y_s per ci-tile, cols 4..7 = bias per co-tile
    yb_ext = nc.declare_dram_parameter("yb", [128, 2 * NCI], F32, isOutput=False)
    # conv weights as per-(jo, a-pair) chunks: [jo, ap, ci_p, jci, a%2, kx, co]
    wa_ext = nc.declare_dram_parameter(
        "wa", [NCO, ALPHA // 2, 128, NCI, 2, 3, 128], F16, isOutput=False
    )
    # demod weights: [jo, ci_p, jci, co]
    w2_ext = nc.declare_dram_parameter(
        "w2", [NCO, 128, NCI, 128], F16, isOutput=False
    )
    out_ext = nc.declare_dram_parameter("out", [NCO, 128, H * W], F16, isOutput=True)

    with TileContext(nc) as tc:
        with (
            tc.tile_pool(name="singles", bufs=1) as singles,
            tc.tile_pool(name="wts", bufs=1) as wts,
            tc.tile_pool(name="pads", bufs=1) as pads,
            tc.tile_pool(name="vts", bufs=1) as vts,
            tc.tile_pool(name="vtmp", bufs=1) as vtmp,
            tc.tile_pool(name="xin", bufs=4) as xin,
            tc.tile_pool(name="mbs", bufs=26) as mbs,
            tc.tile_pool(name="zts", bufs=2) as zts,
            tc.tile_pool(name="outs", bufs=2) as outs,
            tc.tile_pool(name="cps", bufs=6, space="PSUM") as cps,
            tc.tile_pool(name="dps", bufs=1, space="PSUM") as dps,
            tc.tile_pool(name="wps", bufs=1, space="PSUM") as wps,
        ):
            # ---- input DMAs ----
            # x0/x1 + yb from sync, x2/x3 from gpsimd — 4 concurrent x
            # transfers land everything by ~4us while the first weight
            # chunks stream alongside.
            xt_sb = [
                xin.tile([128, H, W], F16, tag=f"x{j}", name=f"xt{j}")
                for j in range(NCI)
            ]
            yb_sb = singles.tile([128, 2 * NCI], F32)
            nc.sync.dma_start(out=xt_sb[0], in_=x_ext[0])
            nc.sync.dma_start(out=yb_sb, in_=yb_ext[:, :])
            nc.sync.dma_start(out=xt_sb[1], in_=x_ext[1])

            # weight chunk stream: per-(jo, a-pair) 786KB chunks whose
            # arrival order matches a-major consumption: (a01: jo0..3),
            # (a23: jo0..3), w2 jo0..3, (a45: jo0..3). 3 transfers kept
            # in flight: enough to hide the ~2us fixed completion
            # latency per DMA, few enough that round-robin doesn't
            # starve the first-needed chunk.
            wa_sb = [[None] * (ALPHA // 2) for _ in range(NCO)]
            w2_sb = [None] * NCO
            chunk_order = []
            for ap in range(2):
                for jo in range(NCO):
                    chunk_order.append((jo, ap))
            for jo in range(NCO):
                chunk_order.append((jo, -1))  # w2
            for jo in range(NCO):
                chunk_order.append((jo, 2))

            def wdma(c):
                jo, ap = c
                if ap < 0:
                    w = wts.tile([128, NCI, 128], F16, tag=f"w2_{jo}", name=f"w2_{jo}")
                    nc.gpsimd.dma_start(out=w, in_=w2_ext[jo])
                    w2_sb[jo] = w
                else:
                    w = wts.tile(
                        [128, NCI, 2, 3, 128], F16,
                        tag=f"wa{jo}_{ap}", name=f"wa{jo}_{ap}",
                    )
                    nc.gpsimd.dma_start(out=w, in_=wa_ext[jo, ap])
                    wa_sb[jo][ap] = w

            def chunk_probe(c):
                jo, ap = c
                if ap < 0:
                    return w2_sb[jo][0:1, 0, 0:1]
                return wa_sb[jo][ap][0:1, 0, 0, 0, 0:1]

            wscr = singles.tile([1, 1], F16)

            wdma(chunk_order[0])
            wdma(chunk_order[1])
            nc.gpsimd.dma_start(out=xt_sb[2], in_=x_ext[2])
            nc.gpsimd.dma_start(out=xt_sb[3], in_=x_ext[3])
            wdma(chunk_order[2])
            for i in range(3, len(chunk_order)):
                nc.gpsimd.tensor_copy(out=wscr, in_=chunk_probe(chunk_order[i - 3]))
                wdma(chunk_order[i])

            def wa_slice(jo, jci, a, kx):
                return wa_sb[jo][a // 2][:, jci, a % 2, kx, :]

            # ---- PE warm-up: throwaway matmuls on memset data so the
            # HAM clock gate starts releasing before the real stream ----
            warm_lhs = singles.tile([128, 1], F16)
            nc.vector.memset(warm_lhs, 1.0)
            warm_rhs = singles.tile([128, 512], F16)
            nc.vector.memset(warm_rhs, 0.5)
            warm_ps = wps.tile([1, 512], F32)
            N_WARM = 8
            for i in range(N_WARM):
                nc.tensor.matmul(
                    out=warm_ps,
                    lhsT=warm_lhs,
                    rhs=warm_rhs,
                    start=(i == 0),
                    stop=(i == N_WARM - 1),
                )

            eps_sb = singles.tile([128, 1], F32)
            nc.vector.memset(eps_sb, EPS_EFF)

            # pad border memsets on DVE (fast, and DVE owns pad deps
            # anyway; gpsimd is busy issuing the weight chain)
            pad_sb = []
            for j in range(NCI):
                p = pads.tile([128, PADH, PADW], F16, tag=f"pad{j}")
                nc.vector.memset(p[:, 0, :], 0.0)
                nc.vector.memset(p[:, PADH - 1, :], 0.0)
                nc.vector.memset(p[:, 1 : PADH - 1, 0:2], 0.0)
                nc.vector.memset(p[:, 1 : PADH - 1, PADW - 2 : PADW], 0.0)
                pad_sb.append(p)

            # modulate on ACT (frees the DVE for the V transform)
            for j in range(NCI):
                nc.scalar.activation(
                    out=pad_sb[j][:, 1 : H + 1, 2 : W + 2],
                    in_=xt_sb[j],
                    func=AF.Identity,
                    scale=yb_sb[:, j : j + 1],
                )

            # ---- F(4,3) y-transform (DVE fp16), emitted a-MAJOR so
            # V levels become ready in consumption order ----
            v_sb = [[None] * NCI for _ in range(ALPHA)]
            for a in range(ALPHA):
                for j in range(NCI):
                    v_sb[a][j] = vts.tile(
                        [128, NTY, PADW], F16, tag=f"v{a}_{j}", name=f"v{a}_{j}"
                    )
            P_ = lambda j, r: pad_sb[j][:, r : r + 29 : 4, :]

            def vtile(nm, j):
                return vtmp.tile([128, NTY, PADW], F16, tag=f"{nm}{j}", name=f"{nm}{j}")

            stt = nc.vector.scalar_tensor_tensor
            tmps = [dict() for _ in range(NCI)]
            for j in range(NCI):  # V0 = 4*P0 - 5*P2 + P4
                t0 = vtile("t0", j)
                stt(out=t0, in0=P_(j, 2), scalar=-5.0, in1=P_(j, 4), op0=ALU.mult, op1=ALU.add)
                stt(out=v_sb[0][j], in0=P_(j, 0), scalar=4.0, in1=t0, op0=ALU.mult, op1=ALU.add)
            for j in range(NCI):  # V1 = -4*(P1+P2) + (P3+P4)
                s12 = vtile("s12", j)
                s34 = vtile("s34", j)
                nc.vector.tensor_add(s12, P_(j, 1), P_(j, 2))
                nc.vector.tensor_add(s34, P_(j, 3), P_(j, 4))
                stt(out=v_sb[1][j], in0=s12, scalar=-4.0, in1=s34, op0=ALU.mult, op1=ALU.add)
            for j in range(NCI):  # V2 = 4*(P1-P2) + (P4-P3)
                d12 = vtile("d12", j)
                d43 = vtile("d43", j)
                nc.vector.tensor_sub(d12, P_(j, 1), P_(j, 2))
                nc.vector.tensor_sub(d43, P_(j, 4), P_(j, 3))
                stt(out=v_sb[2][j], in0=d12, scalar=4.0, in1=d43, op0=ALU.mult, op1=ALU.add)
            for j in range(NCI):  # V3 = 2*(P3-P1) + (P4-P2)
                d31 = vtile("d31", j)
                d42 = vtile("d42", j)
                nc.vector.tensor_sub(d31, P_(j, 3), P_(j, 1))
                nc.vector.tensor_sub(d42, P_(j, 4), P_(j, 2))
                stt(out=v_sb[3][j], in0=d31, scalar=2.0, in1=d42, op0=ALU.mult, op1=ALU.add)
                tmps[j]["d31"] = d31
                tmps[j]["d42"] = d42
            for j in range(NCI):  # V4 = -2*(P3-P1) + (P4-P2)
                stt(out=v_sb[4][j], in0=tmps[j]["d31"], scalar=-2.0, in1=tmps[j]["d42"], op0=ALU.mult, op1=ALU.add)
            for j in range(NCI):  # V5 = 4*P1 - 5*P3 + P5
                t5 = vtile("t5", j)
                stt(out=t5, in0=P_(j, 3), scalar=-5.0, in1=P_(j, 5), op0=ALU.mult, op1=ALU.add)
                stt(out=v_sb[5][j], in0=P_(j, 1), scalar=4.0, in1=t5, op0=ALU.mult, op1=ALU.add)

            # ys^2 in fp16 for the demod matmuls
            ys2_sb = singles.tile([128, NCI], F16)
            nc.vector.tensor_mul(ys2_sb, yb_sb[:, 0:NCI], yb_sb[:, 0:NCI])

            xs2_ps = dps.tile([128, NCO], F32)
            rs_sb = singles.tile([128, NCO], F32)
            mb = [[None] * ALPHA for _ in range(NCO)]

            def epilogue(jo):
                # demod
                for jci in range(NCI):
                    nc.tensor.matmul(
                        out=xs2_ps[:, jo : jo + 1],
                        lhsT=w2_sb[jo][:, jci, :],
                        rhs=ys2_sb[:, jci : jci + 1],
                        start=(jci == 0),
                        stop=(jci == NCI - 1),
                    )
                nc.scalar.activation(
                    out=rs_sb[:, jo : jo + 1],
                    in_=xs2_ps[:, jo : jo + 1],
                    func=AF.Sqrt,
                    bias=eps_sb,
                )
                nc.vector.reciprocal(
                    out=rs_sb[:, jo : jo + 1], in_=rs_sb[:, jo : jo + 1]
                )
                # combine (DVE fp16):
                #   p0 = M0+M1+M2+M3+M4, p1 = (M1-M2) + 2(M3-M4)
                #   p2 = (M1+M2) + 4(M3+M4), p3 = ((M1-M2)+M5) + 8(M3-M4)
                def zt(tag):
                    return zts.tile([128, NTY, W], F16, tag=tag, name=tag)

                m = mb[jo]
                sp, sm, tp, tm = zt("sp"), zt("sm"), zt("tp"), zt("tm")
                nc.vector.tensor_add(sp, m[1], m[2])
                nc.vector.tensor_sub(sm, m[1], m[2])
                nc.vector.tensor_add(tp, m[3], m[4])
                nc.vector.tensor_sub(tm, m[3], m[4])
                u0, z0, z1, z2, z3, u3 = (
                    zt("u0"), zt("z0"), zt("z1"), zt("z2"), zt("z3"), zt("u3"),
                )
                nc.vector.tensor_add(u0, m[0], sp)
                nc.vector.tensor_add(z0, u0, tp)
                stt(out=z1, in0=tm, scalar=2.0, in1=sm, op0=ALU.mult, op1=ALU.add)
                stt(out=z2, in0=tp, scalar=4.0, in1=sp, op0=ALU.mult, op1=ALU.add)
                nc.vector.tensor_add(u3, sm, m[5])
                stt(out=z3, in0=tm, scalar=8.0, in1=u3, op0=ALU.mult, op1=ALU.add)
                # out rows 4ty+p = Z_p * rs + bias
                ot = outs.tile([128, H, W], F16, tag="ot", name="ot")
                for p, z in ((0, z0), (1, z1), (2, z2), (3, z3)):
                    nc.scalar.activation(
                        out=ot[:, p : p + 29 : 4, :],
                        in_=z,
                        func=AF.Identity,
                        bias=yb_sb[:, NCI + jo : NCI + jo + 1],
                        scale=rs_sb[:, jo : jo + 1],
                    )
                nc.sync.dma_start(out=out_ext[jo], in_=ot)

            # ---- main stream: a-major groups; per-jo epilogues are
            # emitted inside the a5 level right after each jo's last
            # group so only jo3's epilogue trails the stream ----
            for a in range(ALPHA):
                for jo in range(NCO):
                    ps = cps.tile([128, NTY, W], F32, tag="m", name="m")
                    idx = 0
                    for jci in range(NCI):
                        for kx in range(3):
                            nc.tensor.matmul(
                                out=ps,
                                lhsT=wa_slice(jo, jci, a, kx),
                                rhs=v_sb[a][jci][:, :, kx + 1 : kx + 1 + W],
                                start=(idx == 0),
                                stop=(idx == 11),
                            )
                            idx += 1
                    m = mbs.tile([128, NTY, W], F16, tag="mb", name="mb")
                    nc.scalar.activation(out=m, in_=ps, func=AF.Copy)
                    mb[jo][a] = m
                    if a == ALPHA - 1:
                        epilogue(jo)

            # keep the warm-up matmuls live (cheap PSUM read at the end)
            warm_sink = singles.tile([1, 1], F32)
            nc.vector.tensor_copy(out=warm_sink, in_=warm_ps[0:1, 0:1])
    nc.compile()
    return nc


_NC_CACHE = None


def _get_nc():
    global _NC_CACHE
    if _NC_CACHE is None:
        _NC_CACHE = build_nc()
    return _NC_CACHE


def _prep_inputs(x, y_s, weight, bias):
    # Winograd weight transform (input-independent): U1[a,kx,ci,co] =
    # sum_ky G[a,ky] w[co,ci,ky,kx]; w2 = sum_k w^2 for demod.
    G = np.array(
        [
            [1 / 4, 0, 0],
            [-1 / 6, -1 / 6, -1 / 6],
            [-1 / 6, 1 / 6, -1 / 6],
            [1 / 24, 1 / 12, 1 / 6],
            [1 / 24, -1 / 12, 1 / 6],
            [0, 0, 1],
        ],
        np.float64,
    )
    w64 = weight.astype(np.float64)  # [co, ci, ky, kx]
    u1 = np.einsum("ag,oigx->axio", G, w64)  # [a, kx, ci, co]
    # -> [jo, ap, ci_p, jci, a%2, kx, co]
    wa = np.ascontiguousarray(
        u1.reshape(3, 2, 3, NCI, 128, NCO, 128).transpose(5, 0, 4, 3, 1, 2, 6)
    ).astype(np.float16)
    w2 = (w64**2).sum(axis=(2, 3)).T  # [ci, co]
    w2q = np.ascontiguousarray(
        w2.reshape(NCI, 128, NCO, 128).transpose(2, 1, 0, 3)
    ).astype(np.float16)
    in_maps = []
    for b in range(B):
        yb = np.empty((128, 2 * NCI), np.float32)
        yb[:, :NCI] = y_s[b].reshape(NCI, 128).T
        yb[:, NCI:] = bias.reshape(NCO, 128).T
        in_maps.append(
            {
                "x": np.ascontiguousarray(x[b].reshape(NCI, 128, H, W)).astype(
                    np.float16
                ),
                "yb": yb,
                "wa": wa,
                "w2": w2q,
            }
        )
    return in_maps


def _install_trace_support():
    """Dev-only: register the axon NTFF profiling hook + disable the
    remote artifact upload so trace=True works in this container."""
    import sys
    import types

    import concourse.bass_utils as bu

    bu.upload_artifacts = lambda tmpdir: "local://" + str(tmpdir)
    if "antenv.axon_hooks" in sys.modules:
        return
    try:
        from trn_agent_boot.trn_boot import _ntff_profile_via_ctypes

        hook = _ntff_profile_via_ctypes("/opt/axon/libaxon_pjrt.so")
    except Exception:
        return
    mod = types.ModuleType("antenv.axon_hooks")
    mod.get_axon_ntff_profile_hook = lambda: hook
    mod.set_axon_ntff_profile_hook = lambda h: None
    sys.modules["antenv.axon_hooks"] = mod


def run(x, y_s, weight, bias, trace=False, tmpdir=None):
    nc = _get_nc()
    if trace:
        _install_trace_support()
    in_maps = _prep_inputs(x, y_s, weight, bias)
    res = run_bass_kernel_spmd(
        nc, in_maps, core_ids=list(range(B)), trace=trace, tmpdir=tmpdir
    )
    out = np.stack(
        [res.results[b]["out"].reshape(CO, H, W).astype(np.float32) for b in range(B)]
    )
    return out, res


def kernel(x, y_s, weight, bias):
    out, _ = run(
        np.asarray(x, dtype=np.float32),
        np.asarray(y_s, dtype=np.float32),
        np.asarray(weight, dtype=np.float32),
        np.asarray(bias, dtype=np.float32),
    )
    return out
